# revision 1
# baseline (speedup 1.0000x reference)
"""Bidirectional-GRU document encoder (BiGRU + additive attention pooling)
for Trainium2, SPMD over 8 NeuronCores.

Sharding: 8 cores = 2 directions x 4 doc-groups (8 docs each). Backward
cores receive time-flipped input from the host, so the device program is
identical on every core (pure SPMD; only the fed data differs per core).

Everything on-device runs in a transposed layout (hidden dim on SBUF
partitions) so the GRU gate element-wise work uses all 128 lanes:
  - input projection:  xw.T = W_ih @ x.T   (big efficient matmuls)
  - recurrence step:   gh.T = W_hh @ h.T   (27 LDW+MM pairs, N=batch)
  - gates: DVE/ACT on [128, *, B] slices, per-partition bias APs
Direction pairs exchange hidden states once at the end via a pairwise
AllGather (own h written time-reversed, so the peer's copy arrives
time-aligned with the local time direction), then each core computes the
full attention scores, softmax, and pools its own direction's half of
the output embedding. The host assembles the [32, 768] result.
"""

import numpy as np
import ml_dtypes

import concourse.bacc as bacc
import concourse.bass as bass
import concourse.mybir as mybir
import concourse.tile as tile
from concourse.bass_utils import run_bass_kernel_spmd

F32 = mybir.dt.float32
BF16 = mybir.dt.bfloat16
AF = mybir.ActivationFunctionType
ALU = mybir.AluOpType
bf16 = ml_dtypes.bfloat16

# Problem constants
B, S, D, H = 32, 512, 768, 384
NCORES = 8
BG = 8                 # docs per core
KD = D // 128          # 6  k-chunks of input dim
M3 = 3 * H // 128      # 9  m-chunks of gate dim
KH = H // 128          # 3  k-chunks of hidden dim
MA = 2 * H // 128      # 6  m-chunks of attention rows


def build_program(steps=S, bg=BG):
    """Build the SPMD Bass program (identical on all 8 cores)."""
    nc = bacc.Bacc("TRN2", target_bir_lowering=False, debug=False,
                   num_devices=NCORES)

    cols = steps * bg                       # size of the (t, b) plane
    ncol = min(512, cols)                   # matmul N-chunk (<= one psum bank)
    nchunks = cols // ncol
    ct = ncol // bg                         # timesteps per N-chunk
    split = nchunks >= 2 and (steps // 2) % ct == 0
    half = steps // 2 if split else steps   # s >= half exchanges early

    # ---- DRAM I/O ----
    xt_d = nc.dram_tensor("xt", [KD, 128, cols], BF16, kind="ExternalInput")
    wih_d = nc.dram_tensor("wih", [M3 * KD, 128, 128], BF16, kind="ExternalInput")
    whh_d = nc.dram_tensor("whh", [M3 * KH, 128, 128], BF16, kind="ExternalInput")
    xwb_d = nc.dram_tensor("xwb", [128, M3], F32, kind="ExternalInput")
    idn_d = nc.dram_tensor("idn", [128, 128], BF16, kind="ExternalInput")
    bnb_d = nc.dram_tensor("bnb", [128, KH, bg], BF16, kind="ExternalInput")
    wao_d = nc.dram_tensor("wao", [MA * KH, 128, 128], BF16, kind="ExternalInput")
    wap_d = nc.dram_tensor("wap", [MA * KH, 128, 128], BF16, kind="ExternalInput")
    bat_d = nc.dram_tensor("bat", [128, MA], F32, kind="ExternalInput")
    ctx_d = nc.dram_tensor("ctx", [128, MA], BF16, kind="ExternalInput")
    doc_d = nc.dram_tensor("doc", [128, KH, bg], F32, kind="ExternalOutput")

    # Internal DRAM: hidden-state exchange (split in two halves so the
    # first AllGather overlaps the recurrence) + small reshape scratch.
    nA = half
    nB = steps - half
    cc_inA = nc.dram_tensor("cc_inA", [128, nA, KH, bg], BF16)
    cc_outA = nc.dram_tensor("cc_outA", [2, 128, nA, KH, bg], BF16)
    if split:
        cc_inB = nc.dram_tensor("cc_inB", [128, nB, KH, bg], BF16)
        cc_outB = nc.dram_tensor("cc_outB", [2, 128, nB, KH, bg], BF16)
    sc_d = nc.dram_tensor("sc_scratch", [1, nchunks, ct, bg], F32)
    at_d = nc.dram_tensor("at_scratch", [bg, steps], BF16)
    groups = [[0, 4], [1, 5], [2, 6], [3, 7]]

    # P1 pieces interleaved into the recurrence: chunk c (c >= 2) piece m
    # is emitted just before step (c - 1) * ct + m * 5.
    pieces = {}
    upfront = 1
    for c in range(upfront, nchunks):
        for m in range(M3):
            pieces.setdefault(max(0, (c - 1) * ct + m * 5 - 8), []).append((c, m))

    with tile.TileContext(nc) as tc:
        with (
            tc.tile_pool(name="const", bufs=1) as cpool,
            tc.tile_pool(name="state", bufs=1) as spool,
            tc.tile_pool(name="work", bufs=3) as wpool,
        ):
            # ---- constants to SBUF ----
            whh = cpool.tile([128, M3 * KH, 128], BF16)
            xwb = cpool.tile([128, M3], F32)
            idn = cpool.tile([128, 128], BF16)
            bnb = cpool.tile([128, KH, bg], BF16)
            wao = cpool.tile([128, MA * KH, 128], BF16)
            wap = cpool.tile([128, MA * KH, 128], BF16)
            bat = cpool.tile([128, MA], F32)
            ctxt = cpool.tile([128, MA], BF16)
            nc.sync.dma_start(whh[:], whh_d[:].rearrange("t p c -> p t c"))
            nc.sync.dma_start(xwb[:], xwb_d[:])
            nc.sync.dma_start(idn[:], idn_d[:])
            nc.sync.dma_start(bnb[:], bnb_d[:])
            nc.sync.dma_start(wao[:], wao_d[:].rearrange("t p c -> p t c"))
            nc.sync.dma_start(wap[:], wap_d[:].rearrange("t p c -> p t c"))
            nc.sync.dma_start(bat[:], bat_d[:])
            nc.sync.dma_start(ctxt[:], ctx_d[:])

            # ---- persistent state ----
            hist16 = spool.tile([128, KH, steps + 1, bg], BF16)
            nc.vector.memset(hist16[:, :, 0, :], 0.0)

            with (
                tc.tile_pool(name="xwp", bufs=1) as xwp,
                tc.tile_pool(name="xin", bufs=1) as xpool,
                tc.tile_pool(name="xtc", bufs=2) as xtp,
                tc.tile_pool(name="ps1", bufs=2,
                             space=bass.MemorySpace.PSUM) as psA,
                tc.tile_pool(name="ps2", bufs=2,
                             space=bass.MemorySpace.PSUM) as psB,
            ):
                xw = xwp.tile([128, M3, steps, bg], BF16)     # xw.T
                wih = xpool.tile([128, M3 * KD, 128], BF16)
                nc.sync.dma_start(wih[:], wih_d[:].rearrange("t p c -> p t c"))

                def xw_chunk_mms(c, ms):
                    csl = slice(c * ncol, (c + 1) * ncol)
                    xtc = xtp.tile([128, KD, ncol], BF16, tag="xtc")
                    for k in range(KD):
                        nc.sync.dma_start(xtc[:, k, :], xt_d[k][:, csl])
                    for m in ms:
                        px = psA.tile([128, ncol], F32, tag="px")
                        for k in range(KD):
                            nc.tensor.matmul(
                                px[:], wih[:, m * KD + k, :], xtc[:, k, :],
                                start=(k == 0), stop=(k == KD - 1))
                        nc.vector.tensor_scalar(
                            out=xw[:, m, c * ct:(c + 1) * ct, :]
                                .rearrange("p t b -> p (t b)"),
                            in0=px[:], scalar1=xwb[:, m:m + 1],
                            scalar2=None, op0=ALU.add)

                # Phase 1 prologue: first chunks so the recurrence can start
                for c in range(upfront):
                    xw_chunk_mms(c, range(M3))

                # ======= Phase 2: GRU recurrence =======
                for t in range(steps):
                    ghrz = psB.tile([128, 6, bg], F32, tag="ghrz")
                    ghn = psB.tile([128, KH, bg], F32, tag="ghn")
                    # seed psum with xw (rz) / bn (n) via identity matmul,
                    # then accumulate the recurrent W_hh terms
                    nc.tensor.matmul(ghrz[:], idn[:], xw[:, 0:6, t, :],
                                     start=True, stop=False)
                    nc.tensor.matmul(ghn[:], idn[:], bnb[:],
                                     start=True, stop=False)
                    for m in range(M3):
                        dst = ghrz[:, m, :] if m < 6 else ghn[:, m - 6, :]
                        for k in range(KH):
                            nc.tensor.matmul(
                                dst, whh[:, m * KH + k, :], hist16[:, k, t, :],
                                start=False,
                                stop=(k == KH - 1 and m in (5, M3 - 1)))
                    # r, z = sigmoid(psum) straight from PSUM
                    rz = wpool.tile([128, 6, bg], F32, tag="rz")
                    nc.scalar.activation(rz[:], ghrz[:], AF.Sigmoid)
                    # n = tanh(xn + r * (hn + bn))
                    t3 = wpool.tile([128, KH, bg], F32, tag="t3")
                    nc.vector.tensor_tensor(
                        out=t3[:], in0=ghn[:], in1=rz[:, 0:3, :], op=ALU.mult)
                    nin = wpool.tile([128, KH, bg], F32, tag="nin")
                    inin = nc.vector.tensor_tensor(
                        out=nin[:], in0=t3[:], in1=xw[:, 6:9, t, :], op=ALU.add)
                    ngate = wpool.tile([128, KH, bg], F32, tag="ngate")
                    nc.scalar.activation(ngate[:], nin[:], AF.Tanh)
                    # h' = n*(1-z) + z*h ; q/zh computed while tanh runs
                    q = wpool.tile([128, KH, bg], F32, tag="q")
                    iq = nc.vector.tensor_scalar(
                        out=q[:], in0=rz[:, 3:6, :], scalar1=-1.0, scalar2=1.0,
                        op0=ALU.mult, op1=ALU.add)
                    zh = wpool.tile([128, KH, bg], F32, tag="zh")
                    izh = nc.vector.tensor_tensor(
                        out=zh[:], in0=rz[:, 3:6, :], in1=hist16[:, :, t, :],
                        op=ALU.mult)
                    # scheduler-only edges: don't let q/zh preempt nin on DVE
                    tile.add_dep_helper(iq.ins, inin.ins, sync=False,
                                        reason="fill tanh window")
                    tile.add_dep_helper(izh.ins, inin.ins, sync=False,
                                        reason="fill tanh window")
                    nq = wpool.tile([128, KH, bg], F32, tag="nq")
                    nc.vector.tensor_tensor(
                        out=nq[:], in0=ngate[:], in1=q[:], op=ALU.mult)
                    nc.vector.tensor_tensor(
                        out=hist16[:, :, t + 1, :], in0=nq[:], in1=zh[:],
                        op=ALU.add)
                    # stash own h time-reversed for the exchange
                    u = steps - 1 - t
                    if split and u >= half:
                        nc.sync.dma_start(cc_inB[:, u - half, :, :],
                                          hist16[:, :, t + 1, :])
                    else:
                        nc.sync.dma_start(cc_inA[:, u, :, :],
                                          hist16[:, :, t + 1, :])
                    for (c, m) in pieces.get(t, ()):   # interleaved P1 work
                        xw_chunk_mms(c, [m])
                    if split and t == half - 1:
                        # upper-s half fully staged: exchange it now
                        nc.gpsimd.collective_compute(
                            "AllGather", ALU.bypass, replica_groups=groups,
                            ins=[cc_inB[:]], outs=[cc_outB[:]])

            # ======= Phase 3: exchange + attention + pooling =======
            ps3 = tc.tile_pool(name="ps3", bufs=5, space=bass.MemorySpace.PSUM)
            psA3 = ps3.__enter__()
            ps3b = tc.tile_pool(name="ps3b", bufs=2,
                                space=bass.MemorySpace.PSUM)
            psB3 = ps3b.__enter__()
            p3s = tc.tile_pool(name="p3s", bufs=1)
            spool3 = p3s.__enter__()
            p3w = tc.tile_pool(name="p3w", bufs=2)
            wpool3 = p3w.__enter__()

            nc.gpsimd.collective_compute(
                "AllGather", ALU.bypass, replica_groups=groups,
                ins=[cc_inA[:]], outs=[cc_outA[:]])
            peer = spool3.tile([128, steps, KH, bg], BF16)

            def resolve_peer(cin, cout, s0, n):
                """peer[:, s0:s0+n] = (slot0 + slot1) - own_reversed."""
                pslice = peer[:, s0:s0 + n, :, :]
                s1t = wpool3.tile([128, n, KH, bg], BF16, tag="s1")
                ownr = wpool3.tile([128, n, KH, bg], BF16, tag="ownr")
                nc.sync.dma_start(pslice, cout[0])
                nc.sync.dma_start(s1t[:], cout[1])
                nc.sync.dma_start(ownr[:], cin[:])
                nc.vector.tensor_tensor(out=pslice, in0=pslice, in1=s1t[:],
                                        op=ALU.add)
                nc.vector.tensor_tensor(out=pslice, in0=pslice, in1=ownr[:],
                                        op=ALU.subtract)

            if split:
                resolve_peer(cc_inB, cc_outB, half, nB)
            resolve_peer(cc_inA, cc_outA, 0, nA)

            # attention scores: sc = ctx . tanh(W_attn @ [own; peer] + b)
            # upper-s chunks first: their peer half resolves first
            order = ([i for i in range(nchunks) if i * ct >= half] +
                     [i for i in range(nchunks) if i * ct < half])
            for nci in order:
                tsl = slice(nci * ct, (nci + 1) * ct)
                psc = psB3.tile([1, ncol], F32, tag="psc")
                for m in range(MA):
                    pa = psA3.tile([128, ncol], F32, tag="pa")
                    for k in range(KH):
                        nc.tensor.matmul(
                            pa[:], wao[:, m * KH + k, :],
                            hist16[:, k, 1 + nci * ct:1 + (nci + 1) * ct, :],
                            start=(k == 0), stop=False)
                    for k in range(KH):
                        nc.tensor.matmul(
                            pa[:], wap[:, m * KH + k, :],
                            peer[:, tsl, k, :],
                            start=False, stop=(k == KH - 1))
                    th = wpool3.tile([128, ncol], BF16, tag="th")
                    nc.scalar.activation(th[:], pa[:], AF.Tanh,
                                         bias=bat[:, m:m + 1])
                    nc.tensor.matmul(psc[:], ctxt[:, m:m + 1], th[:],
                                     start=(m == 0), stop=(m == MA - 1))
                scev = wpool3.tile([1, ncol], F32, tag="scev")
                nc.vector.tensor_copy(scev[:], psc[:])
                nc.sync.dma_start(
                    sc_d[0, nci].unsqueeze(0),
                    scev[:].rearrange("o (t b) -> o t b", t=ct))

            # reshape scores to [bg, steps] via DRAM, then softmax over steps
            sc = spool3.tile([bg, steps], F32)
            nc.sync.dma_start(sc[:].rearrange("b (n t) -> b n t", n=nchunks),
                              sc_d[0].rearrange("n t b -> b n t"))
            negmax = wpool3.tile([bg, 1], F32, tag="negmax")
            nc.vector.reduce_max(negmax[:], sc[:], axis=mybir.AxisListType.X,
                                 negate=True)
            esc = wpool3.tile([bg, steps], F32, tag="esc")
            ssum = wpool3.tile([bg, 1], F32, tag="ssum")
            nc.scalar.activation(esc[:], sc[:], AF.Exp, bias=negmax[:],
                                 accum_out=ssum[:])
            rsum = wpool3.tile([bg, 1], F32, tag="rsum")
            nc.vector.reciprocal(rsum[:], ssum[:])
            attn = spool3.tile([bg, steps], BF16)
            nc.vector.tensor_scalar(out=attn[:], in0=esc[:], scalar1=rsum[:],
                                    scalar2=None, op0=ALU.mult)
            # broadcast attn to all partitions as [128, (b, t)] via DRAM
            nc.sync.dma_start(at_d[:], attn[:])
            attn_bc = spool3.tile([128, bg, steps], BF16)
            nc.sync.dma_start(attn_bc[:],
                              at_d[:].unsqueeze(0).broadcast_to(
                                  [128, bg, steps]))

            # pooling: doc.T[p, c, b] = sum_t h.T[p, c, t, b] * attn[b, t]
            doc = spool3.tile([128, KH, bg], F32)
            with tc.tile_pool(name="poolw", bufs=1) as ppool:
                for c in range(KH):
                    wprod = ppool.tile([128, bg, steps], BF16, tag="wprod")
                    nc.vector.tensor_tensor(
                        out=wprod[:],
                        in0=hist16[:, c, 1:, :].rearrange("p t b -> p b t"),
                        in1=attn_bc[:], op=ALU.mult)
                    nc.vector.reduce_sum(doc[:, c, :], wprod[:],
                                         axis=mybir.AxisListType.X)
            nc.sync.dma_start(doc_d[:], doc[:])
            p3w.__exit__(None, None, None)
            p3s.__exit__(None, None, None)
            ps3b.__exit__(None, None, None)
            ps3.__exit__(None, None, None)

    nc.compile()
    return nc


def _tiles(w, kc, mc):
    """w: [kc*128, mc*128] -> [mc*kc, 128, 128] lhsT tiles, m-major."""
    out = np.empty((mc * kc, 128, 128), dtype=w.dtype)
    for m in range(mc):
        for k in range(kc):
            out[m * kc + k] = w[k * 128:(k + 1) * 128, m * 128:(m + 1) * 128]
    return out


def host_prep(inputs, steps=S, bg=BG):
    """Build the 8 per-core input maps (all host-side numpy)."""
    ip = np.asarray(inputs["ip"], np.float32)[:, :steps, :]
    W_attn = np.asarray(inputs["W_attn"], np.float32)
    b_attn = np.asarray(inputs["b_attn"], np.float32)
    ctx = np.asarray(inputs["context"], np.float32)
    maps = []
    for core in range(NCORES):
        fwd = core < 4
        g = core % 4
        x = ip[g * bg:(g + 1) * bg]              # [bg, steps, D]
        if not fwd:
            x = x[:, ::-1, :]
        sfx = "f" if fwd else "b"
        W_ih = np.asarray(inputs[f"W_ih_{sfx}"], np.float32)
        W_hh = np.asarray(inputs[f"W_hh_{sfx}"], np.float32)
        b_ih = np.asarray(inputs[f"b_ih_{sfx}"], np.float32)
        b_hh = np.asarray(inputs[f"b_hh_{sfx}"], np.float32)

        xt = np.ascontiguousarray(x.transpose(2, 1, 0))     # [D, steps, bg]
        xt = xt.reshape(KD, 128, steps * bg)
        bias = b_ih + np.concatenate([b_hh[:2 * H], np.zeros(H, np.float32)])
        own = slice(0, H) if fwd else slice(H, 2 * H)
        pr = slice(H, 2 * H) if fwd else slice(0, H)
        m = {
            "xt": xt.astype(bf16),
            "wih": _tiles(W_ih.T.astype(bf16), KD, M3),
            "whh": _tiles(W_hh.T.astype(bf16), KH, M3),
            "xwb": np.ascontiguousarray(bias.reshape(M3, 128).T),
            "idn": np.eye(128, dtype=np.float32).astype(bf16),
            "bnb": np.repeat(
                np.ascontiguousarray(b_hh[2 * H:].reshape(KH, 128).T)
                .astype(bf16)[:, :, None], BG, axis=2),
            "wao": _tiles(np.ascontiguousarray(W_attn[:, own].T).astype(bf16),
                          KH, MA),
            "wap": _tiles(np.ascontiguousarray(W_attn[:, pr].T).astype(bf16),
                          KH, MA),
            "bat": np.ascontiguousarray(b_attn.reshape(MA, 128).T),
            "ctx": np.ascontiguousarray(ctx.reshape(MA, 128).T).astype(bf16),
        }
        maps.append(m)
    return maps


def assemble(results, steps=S, bg=BG):
    """Per-core doc tiles [128, KH, bg] -> full [B, 2H] f32."""
    doc = np.zeros((B, 2 * H), np.float32)
    for core in range(NCORES):
        g = core % 4
        half = slice(0, H) if core < 4 else slice(H, 2 * H)
        d = np.asarray(results[core]["doc"])     # [128, KH, bg]
        doc[g * bg:(g + 1) * bg, half] = d.transpose(2, 1, 0).reshape(bg, H)
    return doc


def kernel(**inputs):
    nc = build_program(S, BG)
    in_maps = host_prep(inputs, S, BG)
    res = run_bass_kernel_spmd(nc, in_maps, list(range(NCORES)))
    return assemble(res.results, S, BG)

